# revision 14
# baseline (speedup 1.0000x reference)
"""AxialAttention3D Trainium2 kernel (v2).

Reference: 3 weight branches (d/h/w) of full global 8-head attention over
the flattened 16^3 = 4096 positions of x (1, 128, 16, 16, 16), dim_head 16;
out = gamma * (out_d + out_h + out_w) + x.

Sharding: core h computes head h of all 3 branches.  The 3 branch-units are
STACKED at partition bands 32u (u = 0, 1, 2):
  qrepA rows 32u..32u+15 = (A/4)*(Wq_u x + bq_u)   [A = 128*log2 e]
  krepA rows 32u..32u+15 =         Wk_u x + bk_u
so one slot = one key-tile t x all 3 units:
  scores trio: 3 row-tiled matmuls (bands 32u) -> ONE bf16 PSUM tile
    [128 keys, 3*512 queries] holding A*s for the 3 units -> the 3 MMs
    share a single rhs fetch stream (same qrepA columns).
  exp: routed per-slot to ACT (exp with scale=1/A, exact) or to DVE
    (tensor_scalar_add of B=16250.4375 written through an int16 bitcast:
    the int16 bits of round(A*s+B) ARE bf16(e^s) -- Schraudolph).  bf16
    PSUM + 16-bit operands put the DVE op in 2x mode.
  attn@V trio: 3 col-tiled matmuls (col groups 32u, M=32) accumulate
    out stripes + a ones-column denominator row into one f32 PSUM bank.
Per-chunk epilogue: dstage copy -> reciprocal of the denom rows ->
log-doubling DMA broadcast -> bf16 normalize muls -> one out-projection
matmul whose weight row 96 carries the folded v-bias/out-bias term
(scaled row 96 is pinned to 1.0), -> DMA out.  Host sums the 8 partial
outputs and adds the residual x.
"""

import numpy as np


def _bf16np():
    import ml_dtypes

    return ml_dtypes.bfloat16


HEADS = 8
DH = 16
C = 128
NCORES = 8

_A = 128.0 / np.log(2.0)          # 184.6650...; PSUM scores hold A*s
_B = 16250.4375                   # 127*128 - 5.5625 (Schraudolph offset)

_FULL = dict(CHUNK=512, NCH=8, MT=32, LAG=2, EPI_DELAY=3, W_A=512, WARM_MM=12, K_DUM=0, N_DUM=512, BURST=12, BURST_EVERY=10**9)
_CACHE = {}


def _patch_tile_drain():
    """walrus in this env rejects >1 sync wait on one instruction; split the
    Tile kernel-tail drain's aggregated waits into one drain per wait."""
    import concourse.mybir as mybir
    from concourse.tile import TileContext, ScopedClock

    if getattr(TileContext, "_drain_split_patched", False):
        return

    def _drain_and_barrier_split(self, tick_clock, wait_clock):
        probe = self.nc.sync.drain()
        wait_clock.add_sem_waits(
            probe.ins, ScopedClock({None: tick_clock.global_clock})
        )
        si = probe.ins.sync_info
        waits = list(si.on_wait) if si is not None else []
        if len(waits) > 1:
            si.on_wait = [waits[0]]
            for w in waits[1:]:
                d = self.nc.sync.drain()
                d.ins.sync_info = mybir.SyncInfo(on_wait=[w], on_update=[])
        self.nc.all_engine_barrier()
        assert self.sems is not None
        popped = self.nc._tile_sem_poison_stack.pop()
        assert popped is self._sem_poison
        self.nc.clear_and_free_semaphores(list(self.sems.allocated().values()))
        self.nc.all_engine_barrier()

    TileContext._drain_and_barrier = _drain_and_barrier_split
    TileContext._drain_split_patched = True


def _split_multi_waits(nc):
    """walrus in this env allows at most ONE sync wait per instruction.
    Hoist extra waits onto same-engine NoOps inserted just before."""
    import concourse.mybir as mybir

    for f in nc.m.functions:
        for bb in f.blocks:
            new = []
            changed = False
            for inst in bb.instructions:
                si = inst.sync_info
                if si is not None and si.on_wait and len(si.on_wait) > 1:
                    waits = list(si.on_wait)
                    for j, w in enumerate(waits[:-1]):
                        nop = mybir.InstNoOp(
                            name=f"{inst.name}-w{j}",
                            engine=inst.engine,
                            sync_info=mybir.SyncInfo(on_wait=[w], on_update=[]),
                            bass_nofuse=True,
                        )
                        new.append(nop)
                    si.on_wait = [waits[-1]]
                    changed = True
                new.append(inst)
            if changed:
                bb.instructions = new


def build_nc(cfg=_FULL, split_waits=True):
    import concourse.bass as bass
    import concourse.mybir as mybir
    from concourse import tile

    _patch_tile_drain()

    f32 = mybir.dt.float32
    bf16 = mybir.dt.bfloat16
    i16 = mybir.dt.int16
    Exp = mybir.ActivationFunctionType.Exp
    Copy = mybir.ActivationFunctionType.Copy

    CHUNK, NCH, MT = cfg["CHUNK"], cfg["NCH"], cfg["MT"]
    LAG, EPI_DELAY = cfg["LAG"], cfg["EPI_DELAY"]
    W_A, WARM_MM = cfg["W_A"], cfg["WARM_MM"]
    K_DUM, N_DUM = cfg["K_DUM"], cfg["N_DUM"]
    BURST, BURST_EVERY = cfg["BURST"], cfg["BURST_EVERY"]
    N = MT * 128
    assert N == CHUNK * NCH
    SUB = CHUNK // 32

    nc = bass.Bass("TRN2", target_bir_lowering=False, debug=False)

    x_d = nc.declare_dram_parameter("x", [C, N], bf16, isOutput=False)
    lqa_d = nc.declare_dram_parameter("lqa", [C, 128], bf16, isOutput=False)
    lka_d = nc.declare_dram_parameter("lka", [C, 128], bf16, isOutput=False)
    bqv_d = nc.declare_dram_parameter("bqv", [C, 1], f32, isOutput=False)
    bkv_d = nc.declare_dram_parameter("bkv", [C, 1], f32, isOutput=False)
    wv3_d = nc.declare_dram_parameter("wv3", [C, 96], bf16, isOutput=False)
    wo_d = nc.declare_dram_parameter("wo", [C, 128], bf16, isOutput=False)
    inits_d = nc.declare_dram_parameter("inits", [C, CHUNK], bf16, isOutput=False)
    y_d = nc.declare_dram_parameter("y", [C, N], f32, isOutput=True)

    with tile.TileContext(nc) as tc:
        with (
            tc.tile_pool(name="persist", bufs=1) as pp,
            tc.tile_pool(name="pt", bufs=6) as ptp,
            tc.tile_pool(name="dn", bufs=2) as dnp,
            tc.tile_pool(name="osb", bufs=2) as osbp,
            tc.tile_pool(name="scp", bufs=2, space="PSUM") as scp,
            tc.tile_pool(name="accp", bufs=1, space="PSUM") as accp,
            tc.tile_pool(name="projp", bufs=1, space="PSUM") as projp,
        ):
            # ---- persistent SBUF ----
            x_sb = pp.tile([C, N], bf16, name="x_sb", tag="x")
            for cidx in range(NCH):
                nc.sync.dma_start(
                    x_sb[:, cidx * CHUNK : (cidx + 1) * CHUNK],
                    x_d[:, cidx * CHUNK : (cidx + 1) * CHUNK],
                )
            lqa = pp.tile([C, 128], bf16, name="lqa_sb", tag="lqa")
            lka = pp.tile([C, 128], bf16, name="lka_sb", tag="lka")
            bqv = pp.tile([C, 1], f32, name="bqv_sb", tag="bqv")
            bkv = pp.tile([C, 1], f32, name="bkv_sb", tag="bkv")
            wv3 = pp.tile([C, 96], bf16, name="wv3_sb", tag="wv3")
            wo = pp.tile([C, 128], bf16, name="wo_sb", tag="wo")
            nc.sync.dma_start(lqa[:], lqa_d[:])
            nc.sync.dma_start(lka[:], lka_d[:])
            nc.sync.dma_start(bqv[:], bqv_d[:])
            nc.sync.dma_start(bkv[:], bkv_d[:])
            nc.sync.dma_start(wv3[:], wv3_d[:])
            nc.sync.dma_start(wo[:], wo_d[:])

            qrepA = pp.tile([C, N], bf16, name="qrepA_sb", tag="qA")
            krepA = pp.tile([C, N], bf16, name="krepA_sb", tag="kA")
            vT = pp.tile([C, 96 * MT], bf16, name="vT_sb", tag="vT")
            scaled = [
                pp.tile([C, CHUNK], bf16, name=f"scaled{p}_sb", tag=f"scl{p}")
                for p in range(2)
            ]
            for p in range(2):
                nc.sync.dma_start(scaled[p][:], inits_d[:])

            warm_l = pp.tile([C, 2], bf16, name="warm_l", tag="wl")
            warm_r = pp.tile([C, CHUNK], bf16, name="warm_r", tag="wr")
            nc.vector.memset(warm_l[:], 0.0)
            nc.vector.memset(warm_r[:], 0.0)

            # zero vT, then pin the per-(t,u) ones columns (softmax denom)
            nc.vector.memset(vT[:], 0.0)
            ones_ap = vT[:].rearrange("p (t u d) -> p t u d", u=3, d=32)[:, :, :, 16]
            nc.vector.memset(ones_ap, 1.0)

            # ---- phase-0 emitters (dripped into the slot pipeline) ----
            def emit_q(cidx):
                cs, ce = cidx * CHUNK, (cidx + 1) * CHUNK
                ps = scp.tile([C, CHUNK], f32, name="qps", tag="sc")
                nc.tensor.matmul(
                    ps[:], lhsT=lqa[:], rhs=x_sb[:, cs:ce], start=True, stop=True
                )
                nc.scalar.add(qrepA[:, cs:ce], ps[:], bqv[:])

            def emit_k(cidx):
                cs, ce = cidx * CHUNK, (cidx + 1) * CHUNK
                ps = scp.tile([C, CHUNK], f32, name="kps", tag="sc")
                nc.tensor.matmul(
                    ps[:], lhsT=lka[:], rhs=x_sb[:, cs:ce], start=True, stop=True
                )
                nc.vector.tensor_scalar_add(krepA[:, cs:ce], ps[:], bkv[:])

            def emit_vt(t):
                ps = scp.tile([C, 96], f32, name="vps", tag="sc")
                nc.tensor.matmul(
                    ps[:],
                    lhsT=x_sb[:, t * 128 : (t + 1) * 128],
                    rhs=wv3[:],
                    start=True,
                    stop=True,
                )
                # copy only the 16 value dims per unit; ones col stays pinned
                dstv = vT[:, 96 * t : 96 * (t + 1)].rearrange(
                    "p (u d) -> p u d", d=32
                )[:, :, 0:16]
                srcv = ps[:].rearrange("p (u d) -> p u d", d=32)[:, :, 0:16]
                nc.vector.tensor_copy(dstv, srcv)

            # ---- steady-state slot machinery ----
            n_slots = NCH * MT
            pt_of = {}
            acc_of = {}
            pending_b = []

            def emit_scores(i):
                cidx, t = divmod(i, MT)
                cs, ce = cidx * CHUNK, (cidx + 1) * CHUNK
                sc = scp.tile([C, 3 * CHUNK], f32, name="sc_ps", tag="sc")
                for u in range(3):
                    nc.tensor.matmul(
                        sc[:, u * CHUNK : (u + 1) * CHUNK],
                        lhsT=krepA[32 * u : 32 * u + 16, t * 128 : (t + 1) * 128],
                        rhs=qrepA[32 * u : 32 * u + 16, cs:ce],
                        start=True,
                        stop=True,
                        tile_position=(32 * u, 0),
                    )
                pt = ptp.tile([C, 3 * CHUNK], bf16, name="pt_sb", tag="pt")
                # split each tile between both engines at a bank boundary:
                # DVE takes the leading cols (Schraudolph) so the next trio's
                # first matmul unblocks after the short DVE op; ACT the rest
                nc.vector.tensor_scalar_add(
                    pt[:, 0:W_A].bitcast(i16), sc[:, 0:W_A], float(_B)
                )
                nc.scalar.activation(
                    pt[:, W_A:], sc[:, W_A:], Exp, scale=float(1.0 / _A)
                )
                pt_of[i] = pt

            def emit_attnv(i):
                cidx, t = divmod(i, MT)
                if t == 0:
                    acc_of[cidx] = accp.tile([C, CHUNK], f32, name="acc_ps", tag="acc")
                acc = acc_of[cidx]
                pt = pt_of.pop(i)
                for u in range(3):
                    nc.tensor.matmul(
                        acc[32 * u : 32 * u + 32, :],
                        lhsT=vT[:, 96 * t + 32 * u : 96 * t + 32 * u + 32],
                        rhs=pt[:, u * CHUNK : (u + 1) * CHUNK],
                        start=(t == 0),
                        stop=(t == MT - 1),
                        tile_position=(0, 32 * u),
                    )
                if t == MT - 1:
                    emit_epilogue_a(cidx)

            def emit_epilogue_a(cidx):
                acc = acc_of.pop(cidx)
                ds = dnp.tile([C, CHUNK], bf16, name="ds_sb", tag="ds")
                nc.vector.tensor_copy(ds[0:96, :], acc[0:96, :])
                dn = dnp.tile([C, SUB], bf16, name="dn_sb", tag="dn")
                rc = dnp.tile([C, SUB], bf16, name="rc_sb", tag="rc")
                for u in range(3):
                    nc.sync.dma_start(
                        dn[32 * u : 32 * u + 32, :],
                        ds[32 * u + 16 : 32 * u + 17, :],
                    )
                with nc.allow_low_precision(reason="softmax denom reciprocal in bf16; 0.4% on a 2e-2 budget"):
                    nc.vector.reciprocal(rc[0:96, :], dn[0:96, :])
                nb = dnp.tile([C, CHUNK], bf16, name="nb_sb", tag="nb")
                for u in range(3):
                    nc.sync.dma_start(
                        nb[32 * u : 32 * u + 1, :], rc[32 * u : 32 * u + 32, :]
                    )
                    for w in (1, 2, 4, 8):
                        nc.sync.dma_start(
                            nb[32 * u + w : 32 * u + 2 * w, :],
                            nb[32 * u : 32 * u + w, :],
                        )
                st = scaled[cidx % 2]
                for u in range(3):
                    nc.vector.tensor_mul(
                        st[32 * u : 32 * u + 16, :],
                        ds[32 * u : 32 * u + 16, :],
                        nb[32 * u : 32 * u + 16, :],
                    )

            def emit_burst(k):
                # dense back-to-back dummy matmuls: trip the HAM clock gate
                # (PE un-throttles only after a ~3.4us fully-busy window)
                if k <= 0:
                    return
                dm = projp.tile([C, CHUNK], f32, name="dum_ps", tag="pj")
                for _ in range(k):
                    nc.tensor.matmul(
                        dm[0:2, :], lhsT=warm_l[:], rhs=warm_r[:],
                        start=True, stop=True,
                    )

            def emit_epilogue_b(cidx):
                cs, ce = cidx * CHUNK, (cidx + 1) * CHUNK
                pj = projp.tile([C, CHUNK], f32, name="pj_ps", tag="pj")
                nc.tensor.matmul(
                    pj[:], lhsT=wo[:], rhs=scaled[cidx % 2][:], start=True, stop=True
                )
                ob = osbp.tile([C, CHUNK], f32, name="ob_sb", tag="ob")
                nc.vector.tensor_copy(ob[:], pj[:])
                nc.sync.dma_start(y_d[:, cs:ce], ob[:])

            # ---- PE warmup spin: ~5us of back-to-back matmuls to flip the
            # HAM clock gate to K=8/8 before the real stream starts ----
            wps = scp.tile([C, CHUNK], f32, name="warm_ps", tag="sc")
            for _ in range(WARM_MM):
                nc.tensor.matmul(
                    wps[0:2, 0:CHUNK], lhsT=warm_l[:], rhs=warm_r[:],
                    start=True, stop=True,
                )

            # ---- phase 0 upfront (PE warm, pipelined through scp + DVE) ----
            for cidx in range(NCH):
                emit_k(cidx)
            for t in range(MT):
                emit_vt(t)
            for cidx in range(NCH):
                emit_q(cidx)

            emit_burst(BURST + 4)

            for i in range(n_slots + LAG + EPI_DELAY + 1):
                while pending_b and pending_b[0][0] <= i:
                    emit_epilogue_b(pending_b.pop(0)[1])
                if i > 0 and i % BURST_EVERY == 0:
                    emit_burst(BURST)
                if i < n_slots:
                    emit_scores(i)
                av = i - LAG
                if 0 <= av < n_slots:
                    emit_attnv(av)
                    if av % MT == MT - 1:
                        pending_b.append((i + EPI_DELAY, av // MT))
            while pending_b:
                emit_epilogue_b(pending_b.pop(0)[1])

    if split_waits:
        _split_multi_waits(nc)
    return nc


def host_prep(inputs, cfg=_FULL):
    """Pack the full problem inputs into per-core input maps."""
    CHUNK, MT = cfg["CHUNK"], cfg["MT"]
    N = MT * 128
    bf = _bf16np()

    x = np.asarray(inputs["x"], dtype=np.float32)
    assert x.shape[0] == 1
    xf = np.ascontiguousarray(x.reshape(C, -1))[:, :N]

    gamma0 = float(np.asarray(inputs["gamma"]).reshape(-1)[0])
    branches = [
        (
            np.asarray(inputs[f"w_qkv_{nm}"], dtype=np.float32),
            np.asarray(inputs[f"b_qkv_{nm}"], dtype=np.float32),
            np.asarray(inputs[f"w_out_{nm}"], dtype=np.float32),
            np.asarray(inputs[f"b_out_{nm}"], dtype=np.float32),
        )
        for nm in ("d", "h", "w")
    ]

    beff_total = np.zeros(C, dtype=np.float64)
    for wqkv, bqkv, wout, bout in branches:
        bv = bqkv[2 * C : 3 * C]
        beff_total += gamma0 * (wout.astype(np.float64) @ bv + bout)
    beff_core = (beff_total / NCORES).astype(np.float32)

    Aq = np.float32(_A * 0.25)

    inits = np.zeros((C, CHUNK), dtype=np.float32)
    inits[96, :] = 1.0

    in_maps = []
    for h in range(NCORES):
        lqa = np.zeros((C, 128), dtype=np.float32)
        lka = np.zeros((C, 128), dtype=np.float32)
        bqv = np.zeros((C, 1), dtype=np.float32)
        bkv = np.zeros((C, 1), dtype=np.float32)
        wv3 = np.zeros((C, 96), dtype=np.float32)
        wo = np.zeros((C, 128), dtype=np.float32)
        for u, (wqkv, bqkv, wout, bout) in enumerate(branches):
            wq = wqkv[h * DH : (h + 1) * DH, :]
            wk = wqkv[C + h * DH : C + (h + 1) * DH, :]
            wv = wqkv[2 * C + h * DH : 2 * C + (h + 1) * DH, :]
            bq = bqkv[h * DH : (h + 1) * DH]
            bk = bqkv[C + h * DH : C + (h + 1) * DH]

            lqa[:, 32 * u : 32 * u + 16] = Aq * wq.T
            lka[:, 32 * u : 32 * u + 16] = wk.T
            bqv[32 * u : 32 * u + 16, 0] = Aq * bq
            bkv[32 * u : 32 * u + 16, 0] = bk
            wv3[:, 32 * u : 32 * u + 16] = wv.T
            wo[32 * u : 32 * u + 16, :] = gamma0 * wout[:, h * DH : (h + 1) * DH].T
        wo[96, :] = beff_core
        m = {
            "x": xf.astype(bf),
            "lqa": lqa.astype(bf),
            "lka": lka.astype(bf),
            "bqv": bqv,
            "bkv": bkv,
            "wv3": wv3.astype(bf),
            "wo": wo.astype(bf),
            "inits": inits.astype(bf),
        }
        in_maps.append(m)
    return in_maps


def gather(results, inputs, cfg=_FULL):
    x = np.asarray(inputs["x"], dtype=np.float32)
    N = cfg["MT"] * 128
    acc = np.zeros((C, N), dtype=np.float32)
    for r in results:
        acc += r["y"]
    out = acc + x.reshape(C, -1)[:, :N]
    return out.reshape(x.shape).astype(np.float32)


def kernel(**inputs) -> np.ndarray:
    from concourse.bass_utils import run_bass_kernel_spmd

    if "nc" not in _CACHE:
        _CACHE["nc"] = build_nc(_FULL)
    nc = _CACHE["nc"]
    in_maps = host_prep(inputs, _FULL)
    res = run_bass_kernel_spmd(nc, in_maps, list(range(NCORES)))
    return gather(res.results, inputs, _FULL)
